# revision 21
# baseline (speedup 1.0000x reference)
"""Trainium2 Bass kernel for nn_ContinuousCritic (permutation-invariant twin critic).

Strategy: pure data parallel over 8 NeuronCores (batch 32768 -> 4096/core).

Host-side folding: the reference's _build_inp permutation-stack is affine in the
raw concatenated input x = [obs, ag, g, act] (77 dims): inp[b,p] = A_p @ x_b + c_p.
So layer 1 of each phi twin becomes, per permutation p,
    h1[b,p] = relu(x_b @ W1eff[t,p] + b1eff[t,p]),
with W1eff = A_p^T @ W1 computed once on the host. The bias rides as an extra
constant-one input row (K 77 -> 78), so no bias op is needed for layer 1.

On-device everything is feature-major ([feat_part, batch_free]) so no transposes:
    psum[m, n] += sum_k lhsT[k, m] * rhs[k, n]   (nc.tensor.matmul)
Layer 2 + perm-sum uses relu(v + b2) = max(v, -b2) + b2: a single fused DVE
scalar_tensor_tensor (op0=max, op1=add) accumulates straight from PSUM into the
perm accumulator; the leftover +6*b2 constant is folded into rho layer-1's bias
(rb1eff = rb1 + 6 * b2 @ R1) on the host.

Matmul operands are float16 (full PE rate; fp8 DoubleRow was measured to break
the 2e-2 rel-err budget) with fp32 PSUM accumulation everywhere.

Perf notes vs the first working version (198us):
 - weights ride in 4 consolidated DMAs (was 27) ordered so tile-0/perm-0
   dependencies land first: kills the startup serialization on the sync queue.
 - rho layer 2 gets a per-twin psum tile drained immediately (no long-lived
   shared tile): removes per-tile psum-pool stalls.
 - perm-0 accumulate uses tensor_scalar max (no zeros tile / memset).
 - the last tile's rho chain is emitted right after each twin's final
   accumulate, shrinking the tail drain.
"""

import numpy as np

# ---- problem constants (hardcoded per spec) --------------------------------
B = 32768
N_CORES = 8
BS = B // N_CORES          # 4096 batch per core
TILE = 1024                # batch tile (2 fp32 PSUM banks)
MMN = 512                  # matmul moving size (one fp32 PSUM bank)
NT = BS // TILE            # 4 batch tiles per core
KIN = 78                   # 77 raw features + constant-one row
KPAD = KIN                 # no K padding (128-pad measured neutral on PE, worse head)
HID = 256
NPERM = 6
DIM_BODY, DIM_OBJ, NB = 10, 15, 3

_PROG = None  # cached program


# ---- host-side math ---------------------------------------------------------

def _perms():
    out = []
    for i in range(NB):
        for j in range(NB):
            if i != j:
                out.append((i, j))
    return out  # [(0,1),(0,2),(1,0),(1,2),(2,0),(2,1)] - matches itertools


def _build_inp_np(obs, ag, g, act):
    """Numpy replica of reference._build_inp."""
    b = obs.shape[0]
    obs_body = obs[:, :DIM_BODY]
    obs_obj = obs[:, DIM_BODY:].reshape(b, NB, DIM_OBJ)
    onehot = np.broadcast_to(np.eye(NB, dtype=obs.dtype), (b, NB, NB))
    feats = np.concatenate([onehot, obs_obj], axis=-1)          # [b,3,18]
    ag_o = ag.reshape(b, NB, NB)                                # OBJ_IDS == reshape
    g_o = g.reshape(b, NB, NB)
    perms = _perms()
    pi = np.array([p[0] for p in perms])
    pj = np.array([p[1] for p in perms])
    body = np.broadcast_to(obs_body[:, None, :], (b, NPERM, DIM_BODY))
    actb = np.broadcast_to(act[:, None, :], (b, NPERM, act.shape[1]))
    inp = np.concatenate([
        ag_o[:, pi], ag_o[:, pj], g_o[:, pi], g_o[:, pj],
        body, feats[:, pi], feats[:, pj], actb], axis=-1)       # [b,6,62]
    return inp


def _affine_maps():
    """inp[b,p] = A[p] @ x_b + c[p] with x = concat(obs, ag, g, act) (77 dims).

    Returns A [6,62,77] (as [77,6,62] transposed view) and c [6,62], float64.
    """
    X = np.concatenate([np.eye(77, dtype=np.float64),
                        np.zeros((1, 77), dtype=np.float64)], axis=0)  # [78,77]
    obs, ag, g, act = X[:, :55], X[:, 55:64], X[:, 64:73], X[:, 73:77]
    inp = _build_inp_np(obs, ag, g, act)   # [78, 6, 62]
    c = inp[77]                            # [6, 62] constant part
    Ax = inp[:77] - c[None]                # [77, 6, 62]; Ax[k,p,f] = A[p,f,k]
    return Ax, c


def _fold_weights(inputs):
    """Host-precompute the packed device weight arrays.

    Returns dict with:
      w1 [78, 3072] f16 : layer-1 chunks, col block (p*2+t)*256
      wb [128, 2052] f16: w2a|w2b|r1a|r1b (4x512, kxm-packed) + r2a|r2b (2x2)
      wc [128, 10]  f32 : negb2a|negb2b|rb1a|rb1b (2 cols each) + rb2a,rb2b
                          (scalars at row 0, cols 8/9)
    """
    Ax, c = _affine_maps()
    f = np.float16
    meta = {}

    w1 = np.zeros((KPAD, 12 * 256), np.float16)
    for t, (w1k, b1k) in enumerate((("phi_w1a", "phi_b1a"), ("phi_w1b", "phi_b1b"))):
        W1 = np.asarray(inputs[w1k], np.float64)   # [62, 256]
        b1 = np.asarray(inputs[b1k], np.float64)   # [256]
        W1eff = np.einsum("kpf,fj->pkj", Ax, W1)   # [6,77,256]
        b1eff = c @ W1 + b1[None]                  # [6,256]
        for p in range(NPERM):
            blk = np.concatenate([W1eff[p], b1eff[p][None]], 0)   # [78,256]
            w1[:KIN, (p * 2 + t) * 256:(p * 2 + t + 1) * 256] = blk.astype(f)
    meta["w1"] = np.ascontiguousarray(w1)

    def pack_kxm(W):  # [256,256] -> [128, 4*128] with col ((kc*2+h)*128+m)
        return (np.asarray(W, np.float64).reshape(2, 128, 2, 128)
                .transpose(1, 0, 2, 3).reshape(128, 512))

    wb = np.zeros((128, 2052), np.float16)
    wb[:, 0:512] = pack_kxm(inputs["phi_w2a"]).astype(f)
    wb[:, 512:1024] = pack_kxm(inputs["phi_w2b"]).astype(f)
    wb[:, 1024:1536] = pack_kxm(inputs["rho_w1a"]).astype(f)
    wb[:, 1536:2048] = pack_kxm(inputs["rho_w1b"]).astype(f)
    # r2 [256] -> [128, 2] col kc
    wb[:, 2048:2050] = np.asarray(inputs["rho_w2a"], f).reshape(2, 128).T
    wb[:, 2050:2052] = np.asarray(inputs["rho_w2b"], f).reshape(2, 128).T
    meta["wb"] = np.ascontiguousarray(wb)

    wc = np.zeros((128, 10), np.float32)
    wc[:, 0:2] = -np.asarray(inputs["phi_b2a"], np.float32).reshape(2, 128).T
    wc[:, 2:4] = -np.asarray(inputs["phi_b2b"], np.float32).reshape(2, 128).T
    for t, (b2k, r1k, rb1k) in enumerate((("phi_b2a", "rho_w1a", "rho_b1a"),
                                          ("phi_b2b", "rho_w1b", "rho_b1b"))):
        b2 = np.asarray(inputs[b2k], np.float64)
        R1 = np.asarray(inputs[r1k], np.float64)
        rb1 = np.asarray(inputs[rb1k], np.float64)
        rb1e = rb1 + NPERM * (b2 @ R1)             # fold the +6*b2 perm-sum constant
        wc[:, 4 + 2 * t:6 + 2 * t] = rb1e.reshape(2, 128).T.astype(np.float32)
    wc[0, 8] = np.float32(inputs["rho_b2a"][0])
    wc[0, 9] = np.float32(inputs["rho_b2b"][0])
    meta["wc"] = np.ascontiguousarray(wc)
    return meta


def _build_xt(inputs):
    """xT [78, B]: rows 0..76 = concat(obs, ag, g, act) transposed, row 77 = ones."""
    x = np.concatenate([inputs["obs"], inputs["ag"], inputs["g"], inputs["act"]],
                       axis=1)                     # [B, 77]
    xt = np.zeros((KPAD, x.shape[0]), np.float16)
    xt[:77] = np.asarray(x, np.float16).T
    xt[77] = 1.0
    return np.ascontiguousarray(xt)


def numpy_forward(inputs):
    """Folded-math forward in numpy (for validating the folding, not the device)."""
    m = _fold_weights(inputs)
    xt = _build_xt(inputs)                          # [78, B]
    qs = []
    for t in range(2):
        acc = np.zeros((256, xt.shape[1]), np.float32)
        negb2 = np.concatenate([m["wc"][:, 2 * t], m["wc"][:, 2 * t + 1]])  # [256]
        for p in range(NPERM):
            w1p = m["w1"][:, (p * 2 + t) * 256:(p * 2 + t + 1) * 256]
            h = np.maximum(w1p.T.astype(np.float32) @ xt.astype(np.float32), 0.0)
            W2 = (m["wb"][:, t * 512:(t + 1) * 512].reshape(128, 2, 2, 128)
                  .transpose(1, 0, 2, 3).reshape(256, 256))
            v = W2.T.astype(np.float32) @ h
            acc += np.maximum(v, negb2[:, None])
        R1 = (m["wb"][:, 1024 + t * 512:1024 + (t + 1) * 512]
              .reshape(128, 2, 2, 128).transpose(1, 0, 2, 3).reshape(256, 256))
        rb1 = np.concatenate([m["wc"][:, 4 + 2 * t], m["wc"][:, 5 + 2 * t]])
        s = np.maximum(R1.T.astype(np.float32) @ acc + rb1[:, None], 0.0)
        R2 = np.concatenate([m["wb"][:, 2048 + 2 * t], m["wb"][:, 2049 + 2 * t]])
        q = R2[None, :].astype(np.float32) @ s + m["wc"][0, 8 + t]
        qs.append(np.ascontiguousarray(q.T))        # [B,1]
    return tuple(qs)


# ---- device program ---------------------------------------------------------

def _build_program():
    import concourse.bacc as bacc
    import concourse.mybir as mybir
    import concourse.tile as tile
    from contextlib import ExitStack

    f32 = mybir.dt.float32
    f16 = mybir.dt.float16
    RELU = mybir.ActivationFunctionType.Relu
    IDENT = mybir.ActivationFunctionType.Identity
    MAX = mybir.AluOpType.max
    ADD = mybir.AluOpType.add

    nc = bacc.Bacc("TRN2", target_bir_lowering=False, debug=False)

    xt_d = nc.dram_tensor("xt", [KPAD, BS], f16, kind="ExternalInput")
    w1_d = nc.dram_tensor("w1", [KPAD, 3072], f16, kind="ExternalInput")
    wb_d = nc.dram_tensor("wb", [128, 2052], f16, kind="ExternalInput")
    wc_d = nc.dram_tensor("wc", [128, 10], f32, kind="ExternalInput")
    q_d = nc.dram_tensor("q", [2, BS], f32, kind="ExternalOutput")

    with tile.TileContext(nc) as tc, ExitStack() as ctx:
        wpool = ctx.enter_context(tc.tile_pool(name="wpool", bufs=1))
        xpool = ctx.enter_context(tc.tile_pool(name="xpool", bufs=1))
        h1pool = ctx.enter_context(tc.tile_pool(name="h1pool", bufs=6))
        accpool = ctx.enter_context(tc.tile_pool(name="accpool", bufs=2))
        spool = ctx.enter_context(tc.tile_pool(name="spool", bufs=2))
        qpool = ctx.enter_context(tc.tile_pool(name="qpool", bufs=4))
        pspool = ctx.enter_context(tc.tile_pool(name="pspool", bufs=2, space="PSUM"))

        # ---- loads, in first-use order -------------------------------------
        # x rides the scalar-engine queue; weights ride the sync queue.
        # PE warm-up: a few matmuls on zeroed SBUF while the first DMAs are in
        # flight — keeps the HAM activity window busy so the real matmuls run
        # at 2.4 GHz instead of the cold 1.2 GHz.
        warm = wpool.tile([128, 512], f16, tag="warm")
        nc.gpsimd.memset(warm[:], 0.0)
        for d in range(6):
            psw = pspool.tile([128, TILE], f32, tag=("psL1", "psL2")[d % 2],
                              bufs=2, name=f"warm_{d}")
            nc.tensor.matmul(psw[:, 0:MMN], warm[:, 0:128], warm[:],
                             start=True, stop=True)

        # x tile 0 gets its own SBUF tile: Tile's write-tracking is
        # tile-granular, so sharing one tile with the bulk-x DMA would gate
        # the first matmul on the whole transfer.
        #
        # DMA waits are per-queue counters ("counter >= number of transfers
        # enqueued so far"), so every dma_start is emitted at the latest
        # program point that still precedes its first reader — otherwise the
        # first matmul waits for the entire weight set.
        x0sb = xpool.tile([KPAD, TILE], f16, tag="xt0", name="xt0")
        nc.scalar.dma_start(x0sb[:], xt_d[:, 0:TILE])
        w1a = wpool.tile([KPAD, 512], f16, tag="w1a")       # perm-0 L1 chunks
        nc.sync.dma_start(w1a[:], w1_d[:, 0:512])
        wcsb = wpool.tile([128, 10], f32, tag="wc")
        nc.sync.dma_start(wcsb[:], wc_d[:])
        # the rest are allocated here, loaded inside the emission loop
        xrsb = xpool.tile([KPAD, BS - TILE], f16, tag="xtr", name="xtr")
        wb1 = wpool.tile([128, 1024], f16, tag="wb1")      # w2a | w2b
        w1b = wpool.tile([KPAD, 2560], f16, tag="w1b")      # perm 1..5 L1 chunks
        wb2 = wpool.tile([128, 1028], f16, tag="wb2")      # r1a | r1b | r2a | r2b

        def deferred_loads(k):
            if k == 0:
                nc.sync.dma_start(wb1[:], wb_d[:, 0:1024])
            elif k == 1:
                nc.sync.dma_start(w1b[:], w1_d[:, 512:3072])
            elif k == 3:
                nc.scalar.dma_start(xrsb[:], xt_d[:, TILE:BS])
                nc.sync.dma_start(wb2[:], wb_d[:, 1024:2052])

        def negb2_ap(t, h):
            return wcsb[:, 2 * t + h:2 * t + h + 1]

        def w1_ap(p, t, h):
            if p == 0:
                return w1a[:, t * 256 + h * 128:t * 256 + (h + 1) * 128]
            c = ((p - 1) * 2 + t) * 256
            return w1b[:, c + h * 128:c + (h + 1) * 128]

        # ---- per-unit work (one perm, one twin, one batch tile) ------------
        # Software pipeline: at step k we emit L1(unit k) then L2(unit k-1),
        # so the PE queue always holds ready matmuls while ACT drains h1 —
        # L2's wait on its own unit's relu never stalls the PE FIFO.
        def x_ap(i, b):
            if i == 0:
                return x0sb[:, b * MMN:(b + 1) * MMN]
            c = (i - 1) * TILE + b * MMN
            return xrsb[:, c:c + MMN]

        def emit_L1(i, p, t):
            h1 = h1pool.tile([128, 2 * TILE], f16, tag="h1", bufs=6)
            for h in range(2):
                ps1 = pspool.tile([128, TILE], f32, tag="psL1", bufs=2)
                for b in range(2):
                    nc.tensor.matmul(
                        ps1[:, b * MMN:(b + 1) * MMN],
                        w1_ap(p, t, h),
                        x_ap(i, b),
                        start=True, stop=True)
                nc.scalar.activation(h1[:, h * TILE:(h + 1) * TILE], ps1[:], RELU)
            return h1

        def emit_L2(p, t, h1, acc):
            # phi layer 2 + fused bias/relu/perm-sum:
            # relu(v + b2) + acc == max(v, -b2) + acc (+6*b2 folded in rho1 bias)
            for h in range(2):
                ps2 = pspool.tile([128, TILE], f32, tag="psL2", bufs=2)
                for kc in range(2):
                    for b in range(2):
                        nc.tensor.matmul(
                            ps2[:, b * MMN:(b + 1) * MMN],
                            wb1[:, t * 512 + (kc * 2 + h) * 128:
                                   t * 512 + (kc * 2 + h + 1) * 128],
                            h1[:, kc * TILE + b * MMN:kc * TILE + (b + 1) * MMN],
                            start=(kc == 0), stop=(kc == 1))
                sl = slice(h * TILE, (h + 1) * TILE)
                if p == 0:
                    nc.vector.tensor_scalar(
                        acc[:, sl], ps2[:], negb2_ap(t, h), None, MAX)
                else:
                    nc.vector.scalar_tensor_tensor(
                        acc[:, sl], ps2[:], negb2_ap(t, h), acc[:, sl],
                        op0=MAX, op1=ADD)

        # ---- rho actions (4 matmuls + 1 drain each), spread across slots ---
        def rho1_half(i, t, h, acc, s):
            ps3 = pspool.tile([128, TILE], f32, tag="psL1", bufs=2)
            for kc in range(2):
                for b in range(2):
                    nc.tensor.matmul(
                        ps3[:, b * MMN:(b + 1) * MMN],
                        wb2[:, t * 512 + (kc * 2 + h) * 128:
                               t * 512 + (kc * 2 + h + 1) * 128],
                        acc[:, kc * TILE + b * MMN:kc * TILE + (b + 1) * MMN],
                        start=(kc == 0), stop=(kc == 1))
            nc.scalar.activation(s[:, h * TILE:(h + 1) * TILE], ps3[:], RELU,
                                 bias=wcsb[:, 4 + 2 * t + h:5 + 2 * t + h])

        def rho2_twin(i, t, s):
            psq = pspool.tile([128, TILE], f32, tag="psL2", bufs=2,
                              name=f"psq_{t}_{i}")
            for kc in range(2):
                for b in range(2):
                    nc.tensor.matmul(
                        psq[0:1, b * MMN:(b + 1) * MMN],
                        wb2[:, 1024 + 2 * t + kc:1024 + 2 * t + kc + 1],
                        s[:, kc * TILE + b * MMN:kc * TILE + (b + 1) * MMN],
                        start=(kc == 0), stop=(kc == 1))
            qt = qpool.tile([1, TILE], f32, tag="q", name=f"qt_{t}_{i}")
            nc.scalar.activation(qt[0:1, :], psq[0:1, :], IDENT,
                                 bias=wcsb[0:1, 8 + t:9 + t])
            nc.sync.dma_start(q_d[t:t + 1, i * TILE:(i + 1) * TILE], qt[0:1, :])

        def rho_actions(i, accs):
            acts = []
            ss = [spool.tile([128, 2 * TILE], f16, tag="s", bufs=2,
                             name=f"s{t}_{i}") for t in range(2)]
            # order: t1's last relu after rho2(t0), so the two q drains don't
            # serialize back-to-back on ACT at the kernel tail
            acts.append(lambda: rho1_half(i, 0, 0, accs[0], ss[0]))
            acts.append(lambda: rho1_half(i, 0, 1, accs[0], ss[0]))
            acts.append(lambda: rho1_half(i, 1, 0, accs[1], ss[1]))
            acts.append(lambda: rho2_twin(i, 0, ss[0]))
            acts.append(lambda: rho1_half(i, 1, 1, accs[1], ss[1]))
            acts.append(lambda: rho2_twin(i, 1, ss[1]))
            return acts

        # ---- main loop: pipelined emission ---------------------------------
        units = [(i, p, t) for i in range(NT) for p in range(NPERM)
                 for t in range(2)]
        accs_by_tile = {}
        prev = None          # (p, t, h1, acc) awaiting its L2
        pend_rho = []        # rho actions of the previous tile
        for k, (i, p, t) in enumerate(units):
            if k % 12 == 0:
                accs_by_tile[i] = [
                    accpool.tile([128, 2 * TILE], f16, tag=f"acc{t_}", bufs=2,
                                 name=f"acc{t_}_{i}")
                    for t_ in range(2)]
            h1 = emit_L1(i, p, t)
            if prev is not None:
                emit_L2(*prev)
            prev = (p, t, h1, accs_by_tile[i][t])
            deferred_loads(k)
            if k % 12 == 0 and k > 0:
                pend_rho = rho_actions(i - 1, accs_by_tile[i - 1])
            elif pend_rho:
                pend_rho.pop(0)()
        emit_L2(*prev)
        for a in rho_actions(NT - 1, accs_by_tile[NT - 1]):
            a()

    nc.compile()
    return nc


def _get_program():
    global _PROG
    if _PROG is None:
        _PROG = _build_program()
    return _PROG


# ---- entry points -----------------------------------------------------------

def run(inputs, trace=False):
    from concourse.bass_utils import run_bass_kernel_spmd

    nc = _get_program()
    m = _fold_weights(inputs)
    xt = _build_xt(inputs)

    in_maps = []
    for c in range(N_CORES):
        im = dict(m)
        im["xt"] = np.ascontiguousarray(xt[:, c * BS:(c + 1) * BS])
        in_maps.append(im)

    res = run_bass_kernel_spmd(nc, in_maps, list(range(N_CORES)), trace=trace)
    q = np.concatenate([res.results[c]["q"] for c in range(N_CORES)],
                       axis=1)                      # [2, B]
    qs = tuple(np.ascontiguousarray(q[t].reshape(B, 1), np.float32)
               for t in range(2))
    return qs, res


def kernel(**inputs):
    inputs = {k: np.asarray(v) for k, v in inputs.items()}
    assert inputs["obs"].shape == (B, 55), inputs["obs"].shape
    qs, _ = run(inputs, trace=False)
    return qs
